# revision 31
# baseline (speedup 1.0000x reference)
"""Trainium2 Bass kernel for nn_ContrastiveLoss (SimCLR-style NT-Xent loss).

Reference computation:
    f = normalize(concat([z1, z2]))            # [2B, D] unit rows
    S = f @ f.T / T                            # [8192, 8192]
    loss = mean_i( logsumexp_j(S[i, :]) - S[i, pos_i] )

Symmetric sharding: S is symmetric, so each core computes only 5 of the
8 column-groups of its 1024-row block (groups 0..4 after rotating the 8
row-groups so the core's own rows are group 0).  The missing column
groups 5,6,7 of row-block b are the transposes of blocks computed by
cores b-3, b-2, b-1, and are recovered as COLUMN sums of the exp'd
blocks g=1..3 (a tiny fp8 DoubleRow ones-matmul per block), exchanged
between cores on the host during the final (cheap) reduction.  This cuts
matmul + exp work to 5/8 and HBM traffic to 10 MB/core.

Operand layout: rows are normalized in row-major bf16 (DVE sum-of-
squares + Quake rsqrt + scale), DMA-xbar transposed as native 2-byte
elements into [dp, db, col] (d = 128*db + dp), then cast to fp8e4.  A
DoubleRow matmul contraction pair (dp, t) maps to d = 256h + 128t + dp,
so BOTH operands slice straight out of the same [128, 4, 1024] fp8 tile
with far-strided (1024B) k-pairs and contiguous columns - the layout the
double-pumped weight/ifmap streams require (byte-interleaved pairs run
at 1 elem/cycle).  NOTE: tensor_tensor_reduce hangs TRN2 hardware (sim
is fine) - all mul-reduces must use affine_mul_reduce.

Per 128-row tile r and group g, the [128, 1024] psum block gets:
  g=0: diag extraction (eye mul-reduce, pre-exp), in-place exp with
       fused row-sum (accum_out).
  g=4: pos-pair extraction (same trick; pos offsets are +4B rows = group
       4 after rotation), in-place exp + row-sum.
  g=1..3: exp written as fp8e4 to SBUF (off-diagonal cosines are within
       +-0.25 whp, so exp(S/T) spans ~[e-4, e4] - inside fp8e4 range)
       + fused row-sum; pairs of row-tiles feed a [128, 2, 512] DR
       ones-matmul accumulating column sums in psum.

Host (f64) assembles denominators across cores:
  den[b] = rowsums_b - exp(diag_b/T) + e^{1/T} + sum_g colsums_{b-g}[g]
  loss   = mean(log(den) - pos/T)
The exact-diagonal substitution cancels the fp8 quantization noise of
the dominant e^{1/T} ~ 1.6e6 softmax term (the rest of a row sums to
~1e4), exactly as in the v1 kernel.  No logsumexp max-subtraction is
needed: sum_j exp() <= ~2e10 fits fp32.
"""

import os
import sys

try:
    import concourse.bass  # noqa: F401
except ImportError:
    for _p in ("/root/.axon_site/_ro/trn_rl_repo", "/opt/trn_rl_repo"):
        if _p not in sys.path and os.path.isdir(_p):
            sys.path.insert(0, _p)

import numpy as np

B = 4096
D = 512
T = 0.07
P = 128
NCORES = 8
R = (2 * B) // NCORES  # 1024 rows per block
G = 8                  # total row/col blocks
NG = 5                 # column groups computed per core (symmetry)
GT = R // P            # 8 row tiles per block
H = 2                  # DR contraction halves (256 each)
DB = D // P            # 4 d-blocks of 128

_NC = None


def _build():
    from contextlib import ExitStack

    import concourse.bacc as bacc
    import concourse.tile as tile
    from concourse import mybir
    from concourse.tile import add_dep_helper

    f32 = mybir.dt.float32
    bf16 = mybir.dt.bfloat16
    f8 = mybir.dt.float8e4
    i32 = mybir.dt.int32
    AFT = mybir.ActivationFunctionType
    EXPF = AFT.Exp
    MUL = mybir.AluOpType.mult
    ADD = mybir.AluOpType.add
    SUB = mybir.AluOpType.subtract
    SHR = mybir.AluOpType.logical_shift_right
    DR = mybir.MatmulPerfMode.DoubleRow

    nc = bacc.Bacc(
        "TRN2", target_bir_lowering=False, debug=False, num_devices=NCORES
    )
    fg = [
        nc.dram_tensor(f"f{k}", [R, D], f32, kind="ExternalInput")
        for k in range(NG)
    ]
    eye = nc.dram_tensor("eye", [P, P], f32, kind="ExternalInput")
    sums_out = nc.dram_tensor("sums", [P, NG * GT], f32, kind="ExternalOutput")
    diag_out = nc.dram_tensor("diag", [P, GT], f32, kind="ExternalOutput")
    pos_out = nc.dram_tensor("pos", [P, GT], f32, kind="ExternalOutput")
    csum_out = nc.dram_tensor("csum", [1, 3 * R], f32, kind="ExternalOutput")

    with ExitStack() as ctx:
        tc = ctx.enter_context(tile.TileContext(nc))
        smalls = ctx.enter_context(tc.tile_pool(name="smalls", bufs=1))
        dumps = ctx.enter_context(tc.tile_pool(name="dumps", bufs=4))
        stats = ctx.enter_context(tc.tile_pool(name="stats", bufs=3))
        zbpool = ctx.enter_context(tc.tile_pool(name="zbpool", bufs=5))
        tbpool = ctx.enter_context(tc.tile_pool(name="tbpool", bufs=2))
        f8pool = ctx.enter_context(tc.tile_pool(name="f8pool", bufs=1))
        e8pool = ctx.enter_context(tc.tile_pool(name="e8pool", bufs=2))
        scrpool = ctx.enter_context(tc.tile_pool(name="scrpool", bufs=3))
        psum = ctx.enter_context(tc.tile_pool(name="psum", bufs=3, space="PSUM"))
        cspool = ctx.enter_context(tc.tile_pool(name="cspool", bufs=1, space="PSUM"))

        sums_sb = smalls.tile([P, NG * GT], f32, tag="sums_sb")
        diag_sb = smalls.tile([P, GT], f32, tag="diag_sb")
        pos_sb = smalls.tile([P, GT], f32, tag="pos_sb")
        csum_sb = smalls.tile([1, 3 * R], f32, tag="csum_sb")
        # bf16 identity: the diag/pos extractions multiply it against the
        # bf16 exp scratch (cast-load via SWDGE).
        eye_sb = smalls.tile([P, P], bf16, tag="eye_sb")
        nc.gpsimd.dma_start(out=eye_sb[:], in_=eye[:, :])
        magic = smalls.tile([P, GT], i32, tag="magic")
        nc.vector.memset(magic[:], 0x5F3759DF)
        # DR stationary all-ones [128, 2, 1] with 16B pair stride.
        ones8 = smalls.tile([P, 2, 16], f8, tag="ones8")
        nc.vector.memset(ones8[:], 1.0)

        def mulsum(in0, in1, accum_col):
            # accum_col[p] = sum_x in0[p,x]*in1[p,x]; main out is a
            # throwaway broadcast AP.  (tensor_tensor_reduce would do the
            # same in one standard op but hangs TRN2 hardware.)
            dummy = dumps.tile([P, 1], f32, tag="dummy")
            return nc.vector.affine_mul_reduce(
                out=dummy.broadcast_to(in0.shape),
                accum_out=accum_col,
                in0=in0,
                in1=in1,
                scale=1.0,
                bias=0.0,
            )

        def rsqrt(invn_dst, ssq):
            # 1/max(sqrt(s), eps) == min(rsqrt(s), 1e12); Quake bit-trick
            # + 2 Newton iterations, all on DVE.
            n = ssq.shape[1]
            h = stats.tile([P, n], i32, tag="h")
            nc.vector.tensor_scalar(h[:], ssq.bitcast(i32), 1, None, op0=SHR)
            y = stats.tile([P, n], f32, tag="y")
            nc.vector.tensor_tensor(y[:].bitcast(i32), magic[:, :n], h[:], op=SUB)
            a = stats.tile([P, n], f32, tag="a")
            for _ in range(2):
                nc.vector.tensor_mul(a[:], y[:], y[:])
                nc.vector.tensor_mul(a[:], a[:], ssq)
                nc.vector.tensor_scalar(a[:], a[:], -0.5, 1.5, op0=MUL, op1=ADD)
                nc.vector.tensor_mul(y[:], y[:], a[:])
            nc.vector.tensor_scalar_min(invn_dst, y[:], 1.0e12)

        load_insts = []
        zbs = {}

        def load_group(g, chunks=2):
            # Pacing: one cast-DMA only sustains ~150 GB/s, so run two
            # chunks in parallel (~300 GB/s, near the 358 HBM cap) and
            # chain chunk n behind chunk n-2 to keep arrival order.
            # Group 0 uses four smaller chunks so its per-chunk prep
            # chain starts sooner.
            zb = zbpool.tile([P, GT, D], bf16, tag="zb")
            at = GT // chunks
            for s in range(chunks):
                ld = nc.gpsimd.dma_start(
                    out=zb[:, s * at : (s + 1) * at, :],
                    in_=fg[g][s * at * P : (s + 1) * at * P, :].rearrange(
                        "(a p) d -> p a d", p=P
                    ),
                )
                n = len(load_insts)
                if n >= 2:
                    add_dep_helper(
                        ld.ins, load_insts[n - 2].ins, reason="pace loads"
                    )
                load_insts.append(ld)
            zbs[g] = zb

        ft8s = {}

        ssqs = {}

        def prep_ssq(g, a, after=None):
            if g not in ssqs:
                ssqs[g] = stats.tile(
                    [P, GT], f32, tag=f"ssq{g % 2}", name=f"ssq_{g}"
                )
            ms = mulsum(zbs[g][:, a, :], zbs[g][:, a, :], ssqs[g][:, a : a + 1])
            if after is not None:
                # Hard ordering hint: without it the Tile scheduler's
                # cost model may slot this ssq ahead of the previous
                # group's scale/cast on the (FIFO, counter-semaphore) DVE
                # queue, inflating every downstream wait threshold.
                add_dep_helper(ms.ins, after.ins, reason="dve order")

        def prep_finish(g, two_queues=False):
            # rsqrt + row scale + transpose + fp8 cast for group g (the
            # 8 ssq mul-reduces were already emitted via prep_ssq).
            zb = zbs.pop(g)
            ssq = ssqs.pop(g)
            invn = stats.tile([P, GT], f32, tag="invn")
            rsqrt(invn[:], ssq[:])
            # Native 2-byte xbar transposes (one per row-tile a, keeping
            # both APs within the 2D-in/3D-out transpose constraint):
            # tb[dp, db, a*128+j] = zb[j, a, 128*db+dp], i.e. F^T with
            # d = 128*db + dp on the partition axis and columns ordered
            # like rows (a*128+j).  Each transpose is emitted right after
            # its row-tile's scale so it starts as early as possible.
            # During startup (ACT idle) they are split over both HWDGE
            # queues (SP + ACT) to halve the serial 8 x ~1.27us queue
            # time; in steady state ACT runs exps, so everything stays on
            # the SP queue.  The fp8 cast runs per column half so the
            # first matmuls only wait on 4 transposes.
            tb = tbpool.tile([P, DB, R], bf16, tag="tb")
            ft8 = f8pool.tile([P, DB, R], f8, tag=f"ft8_{g}", name=f"ft8_{g}")
            for half in range(2):
                for a in range(4 * half, 4 * half + 4):
                    nc.vector.tensor_scalar_mul(
                        zb[:, a, :], zb[:, a, :], invn[:, a : a + 1]
                    )
                    q = nc.scalar if (two_queues and a % 2 == 1) else nc.sync
                    q.dma_start(
                        out=tb[:, :, a * P : (a + 1) * P],
                        in_=zb[:, a, :],
                        transpose=True,
                    )
                sl = slice(half * 512, half * 512 + 512)
                nc.vector.tensor_copy(ft8[:, :, sl], tb[:, :, sl])
            ft8s[g] = ft8

        def prep_group0():
            # Startup-critical fast path for group 0: the load arrives in
            # four 2-row-tile chunks (2-wide ~300 GB/s), and each chunk
            # runs its ssq -> rsqrt-slice -> scale -> transpose chain the
            # moment it lands, with the fp8 cast per column half.  This
            # gets the first matmul going ~20us earlier than the serial
            # load-all-then-prep-all chain.
            zb = zbs.pop(0)
            ssq = stats.tile([P, GT], f32, tag="ssq0", name="ssq_0")
            invn = stats.tile([P, GT], f32, tag="invn")
            tb = tbpool.tile([P, DB, R], bf16, tag="tb")
            ft8 = f8pool.tile([P, DB, R], f8, tag="ft8_0", name="ft8_0")
            for c in range(4):
                for a in (2 * c, 2 * c + 1):
                    mulsum(zb[:, a, :], zb[:, a, :], ssq[:, a : a + 1])
                sl2 = slice(2 * c, 2 * c + 2)
                rsqrt(invn[:, sl2], ssq[:, sl2])
                for a in (2 * c, 2 * c + 1):
                    nc.vector.tensor_scalar_mul(
                        zb[:, a, :], zb[:, a, :], invn[:, a : a + 1]
                    )
                    q = nc.scalar if a % 2 == 1 else nc.sync
                    q.dma_start(
                        out=tb[:, :, a * P : (a + 1) * P],
                        in_=zb[:, a, :],
                        transpose=True,
                    )
                if c % 2 == 1:
                    sl = slice((c // 2) * 512, (c // 2) * 512 + 512)
                    cast_i = nc.vector.tensor_copy(ft8[:, :, sl], tb[:, :, sl])
            ft8s[0] = ft8
            return cast_i

        def prep_group(g, two_queues=False, after=None):
            for a in range(GT):
                prep_ssq(g, a, after=after if a == 0 else None)
            prep_finish(g, two_queues)

        def sim_phase(g, prep_g=None):
            # prep_g: group whose ssq mul-reduces are drip-fed one per
            # row-tile into the DVE queue (between this phase's psum
            # extractions), with the rsqrt/scale/transpose/cast tail
            # emitted after the phase - ready one full phase before use.
            ft8g = ft8s[g]
            ft80 = ft8s[0]
            cs = None
            if g in (1, 2, 3):
                cs = cspool.tile([P, R], f32, tag="cs")
            e8 = None
            for r in range(GT):
                ps = psum.tile([P, R], f32, tag="ps")
                for h in range(H):
                    lhsT = ft80[:, 2 * h : 2 * h + 2, r * P : (r + 1) * P]
                    for ns in range(2):
                        nc.tensor.matmul(
                            ps[:, ns * 512 : (ns + 1) * 512],
                            lhsT,
                            ft8g[:, 2 * h : 2 * h + 2, ns * 512 : (ns + 1) * 512],
                            start=(h == 0),
                            stop=(h == H - 1),
                            perf_mode=DR,
                        )
                acc = sums_sb[:, g * GT + r : g * GT + r + 1]
                if g in (1, 2, 3):
                    if r % 2 == 0:
                        e8 = e8pool.tile([P, 2, R], f8, tag="e8")
                    nc.scalar.activation(
                        e8[:, r % 2, :], ps[:], EXPF, scale=1.0 / T, accum_out=acc
                    )
                    if r % 2 == 1:
                        pr = r // 2
                        for ns in range(2):
                            nc.tensor.matmul(
                                cs[0:1, ns * 512 : (ns + 1) * 512],
                                ones8[:, :, 0:1],
                                e8[:, :, ns * 512 : (ns + 1) * 512],
                                start=(pr == 0),
                                stop=(pr == GT // 2 - 1),
                                perf_mode=DR,
                            )
                else:
                    scr = scrpool.tile([P, R], bf16, tag="scr")
                    nc.scalar.activation(
                        scr[:], ps[:], EXPF, scale=1.0 / T, accum_out=acc
                    )
                    # Diag/pos extraction reads the bf16 exp SCRATCH (not
                    # PSUM): psum release then depends on the exp alone,
                    # and the extraction can lag freely (it only feeds the
                    # final host outputs).  bf16 rounding of the dominant
                    # e^{1/T} term leaves a +-0.4% residual on ~1% of the
                    # denominator - ~5e-4 relative on the loss after
                    # averaging, well inside the 2e-2 gate.  Host gets
                    # exp-domain values (see run()).
                    dst = diag_sb if g == 0 else pos_sb
                    mulsum(
                        scr[:, r * P : (r + 1) * P],
                        eye_sb[:],
                        dst[:, r : r + 1],
                    )
                if prep_g is not None:
                    prep_ssq(prep_g, r)
            if g in (1, 2, 3):
                nc.vector.tensor_copy(
                    csum_sb[0:1, (g - 1) * R : g * R], cs[0:1, :]
                )
            if prep_g is not None:
                prep_finish(prep_g)

        # Startup: groups 0 and 1 loaded+prepped before phase 0 (using
        # both HWDGE queues for the transposes while ACT is idle); later
        # groups' loads start immediately and their DVE prep is drip-fed
        # through the preceding phases.
        load_group(0, chunks=4)
        load_group(1)
        cast0 = prep_group0()
        prep_group(1, two_queues=True, after=cast0)
        for g in range(NG):
            if g + 2 < NG:
                load_group(g + 2)
            sim_phase(g, prep_g=g + 2 if g + 2 < NG else None)

        nc.sync.dma_start(out=sums_out[:], in_=sums_sb[:])
        nc.sync.dma_start(out=diag_out[:], in_=diag_sb[:])
        nc.sync.dma_start(out=pos_out[:], in_=pos_sb[:])
        nc.sync.dma_start(out=csum_out[:, :], in_=csum_sb[0:1, :])

    nc.compile()
    return nc


def _get_nc():
    global _NC
    if _NC is None:
        _NC = _build()
    return _NC


def run(z1, z2, trace=False):
    """Run the SPMD kernel; returns (loss, BassKernelResults)."""
    from concourse.bass_utils import run_bass_kernel_spmd

    z1 = np.ascontiguousarray(z1, dtype=np.float32)
    z2 = np.ascontiguousarray(z2, dtype=np.float32)
    F = np.concatenate([z1, z2], axis=0)  # [8192, 512]
    eye_np = np.eye(P, dtype=np.float32)
    in_maps = []
    for c in range(NCORES):
        m = {"eye": eye_np}
        for k in range(NG):
            blk = (c + k) % G
            m[f"f{k}"] = F[blk * R : (blk + 1) * R]
        in_maps.append(m)
    res = run_bass_kernel_spmd(
        _get_nc(), in_maps, core_ids=list(range(NCORES)), trace=trace
    )
    e_diag_true = np.exp(1.0 / T)
    # Per-core row-major [1024] views; row i = rt*128 + p.
    RS, DG, PS, CSa = [], [], [], []
    for r in res.results:
        sums = r["sums"].astype(np.float64)  # [P, NG*GT]
        RS.append(sums.reshape(P, NG, GT).sum(axis=1).T.reshape(R))
        DG.append(r["diag"].astype(np.float64).T.reshape(R))
        PS.append(r["pos"].astype(np.float64).T.reshape(R))
        CSa.append(r["csum"].astype(np.float64).reshape(3, R))  # row g-1
    total = 0.0
    for b in range(G):
        # DG/PS hold exp-domain extractions: exp(diag/T), exp(pos/T).
        den = RS[b] - DG[b] + e_diag_true
        for g in (1, 2, 3):
            den = den + CSa[(b - g) % G][g - 1]
        total += (np.log(den) - np.log(PS[b])).sum()
    loss = total / (2.0 * B)
    return np.float32(loss), res


def kernel(z1, z2, labels=None, **_ignored):
    loss, _ = run(z1, z2, trace=False)
    return np.asarray(loss, dtype=np.float32)


if __name__ == "__main__":
    rng = np.random.default_rng(0)
    a = rng.standard_normal((B, D)).astype(np.float32)
    b = rng.standard_normal((B, D)).astype(np.float32)
    print(kernel(a, b, None))


# revision 32
# speedup vs baseline: 1.0434x; 1.0434x over previous
"""Trainium2 Bass kernel for nn_ContrastiveLoss (SimCLR-style NT-Xent loss).

Reference computation:
    f = normalize(concat([z1, z2]))            # [2B, D] unit rows
    S = f @ f.T / T                            # [8192, 8192]
    loss = mean_i( logsumexp_j(S[i, :]) - S[i, pos_i] )

Symmetric sharding: S is symmetric, so each core computes only 5 of the
8 column-groups of its 1024-row block (groups 0..4 after rotating the 8
row-groups so the core's own rows are group 0).  The missing column
groups 5,6,7 of row-block b are the transposes of blocks computed by
cores b-3, b-2, b-1, and are recovered as COLUMN sums of the exp'd
blocks g=1..3 (a tiny fp8 DoubleRow ones-matmul per block), exchanged
between cores on the host during the final (cheap) reduction.  This cuts
matmul + exp work to 5/8 and HBM traffic to 10 MB/core.

Operand layout: rows are normalized in row-major bf16 (DVE sum-of-
squares + Quake rsqrt + scale), DMA-xbar transposed as native 2-byte
elements into [dp, db, col] (d = 128*db + dp), then cast to fp8e4.  A
DoubleRow matmul contraction pair (dp, t) maps to d = 256h + 128t + dp,
so BOTH operands slice straight out of the same [128, 4, 1024] fp8 tile
with far-strided (1024B) k-pairs and contiguous columns - the layout the
double-pumped weight/ifmap streams require (byte-interleaved pairs run
at 1 elem/cycle).  NOTE: tensor_tensor_reduce hangs TRN2 hardware (sim
is fine) - all mul-reduces must use affine_mul_reduce.

Per 128-row tile r and group g, the [128, 1024] psum block gets:
  g=0: diag extraction (eye mul-reduce, pre-exp), in-place exp with
       fused row-sum (accum_out).
  g=4: pos-pair extraction (same trick; pos offsets are +4B rows = group
       4 after rotation), in-place exp + row-sum.
  g=1..3: exp written as fp8e4 to SBUF (off-diagonal cosines are within
       +-0.25 whp, so exp(S/T) spans ~[e-4, e4] - inside fp8e4 range)
       + fused row-sum; pairs of row-tiles feed a [128, 2, 512] DR
       ones-matmul accumulating column sums in psum.

Host (f64) assembles denominators across cores:
  den[b] = rowsums_b - exp(diag_b/T) + e^{1/T} + sum_g colsums_{b-g}[g]
  loss   = mean(log(den) - pos/T)
The exact-diagonal substitution cancels the fp8 quantization noise of
the dominant e^{1/T} ~ 1.6e6 softmax term (the rest of a row sums to
~1e4), exactly as in the v1 kernel.  No logsumexp max-subtraction is
needed: sum_j exp() <= ~2e10 fits fp32.
"""

import os
import sys

try:
    import concourse.bass  # noqa: F401
except ImportError:
    for _p in ("/root/.axon_site/_ro/trn_rl_repo", "/opt/trn_rl_repo"):
        if _p not in sys.path and os.path.isdir(_p):
            sys.path.insert(0, _p)

import numpy as np

B = 4096
D = 512
T = 0.07
P = 128
NCORES = 8
R = (2 * B) // NCORES  # 1024 rows per block
G = 8                  # total row/col blocks
NG = 5                 # column groups computed per core (symmetry)
GT = R // P            # 8 row tiles per block
H = 2                  # DR contraction halves (256 each)
DB = D // P            # 4 d-blocks of 128

_NC = None


def _build():
    from contextlib import ExitStack

    import concourse.bacc as bacc
    import concourse.tile as tile
    from concourse import mybir
    from concourse.tile import add_dep_helper

    f32 = mybir.dt.float32
    bf16 = mybir.dt.bfloat16
    f8 = mybir.dt.float8e4
    i32 = mybir.dt.int32
    AFT = mybir.ActivationFunctionType
    EXPF = AFT.Exp
    MUL = mybir.AluOpType.mult
    ADD = mybir.AluOpType.add
    SUB = mybir.AluOpType.subtract
    SHR = mybir.AluOpType.logical_shift_right
    DR = mybir.MatmulPerfMode.DoubleRow

    nc = bacc.Bacc(
        "TRN2", target_bir_lowering=False, debug=False, num_devices=NCORES
    )
    fg = [
        nc.dram_tensor(f"f{k}", [R, D], f32, kind="ExternalInput")
        for k in range(NG)
    ]
    eye = nc.dram_tensor("eye", [P, P], f32, kind="ExternalInput")
    sums_out = nc.dram_tensor("sums", [P, NG * GT], f32, kind="ExternalOutput")
    diag_out = nc.dram_tensor("diag", [P, GT], f32, kind="ExternalOutput")
    pos_out = nc.dram_tensor("pos", [P, GT], f32, kind="ExternalOutput")
    csum_out = nc.dram_tensor("csum", [1, 3 * R], f32, kind="ExternalOutput")

    with ExitStack() as ctx:
        tc = ctx.enter_context(tile.TileContext(nc))
        smalls = ctx.enter_context(tc.tile_pool(name="smalls", bufs=1))
        dumps = ctx.enter_context(tc.tile_pool(name="dumps", bufs=4))
        stats = ctx.enter_context(tc.tile_pool(name="stats", bufs=3))
        zbpool = ctx.enter_context(tc.tile_pool(name="zbpool", bufs=5))
        tbpool = ctx.enter_context(tc.tile_pool(name="tbpool", bufs=2))
        f8pool = ctx.enter_context(tc.tile_pool(name="f8pool", bufs=1))
        e8pool = ctx.enter_context(tc.tile_pool(name="e8pool", bufs=2))
        scrpool = ctx.enter_context(tc.tile_pool(name="scrpool", bufs=3))
        psum = ctx.enter_context(tc.tile_pool(name="psum", bufs=3, space="PSUM"))
        cspool = ctx.enter_context(tc.tile_pool(name="cspool", bufs=1, space="PSUM"))

        sums_sb = smalls.tile([P, NG * GT], f32, tag="sums_sb")
        diag_sb = smalls.tile([P, GT], f32, tag="diag_sb")
        pos_sb = smalls.tile([P, GT], f32, tag="pos_sb")
        csum_sb = smalls.tile([1, 3 * R], f32, tag="csum_sb")
        # bf16 identity: the diag/pos extractions multiply it against the
        # bf16 exp scratch (cast-load via SWDGE).
        eye_sb = smalls.tile([P, P], bf16, tag="eye_sb")
        nc.gpsimd.dma_start(out=eye_sb[:], in_=eye[:, :])
        magic = smalls.tile([P, GT], i32, tag="magic")
        nc.vector.memset(magic[:], 0x5F3759DF)
        # DR stationary all-ones [128, 2, 1] with 16B pair stride.
        ones8 = smalls.tile([P, 2, 16], f8, tag="ones8")
        nc.vector.memset(ones8[:], 1.0)

        def mulsum(in0, in1, accum_col):
            # accum_col[p] = sum_x in0[p,x]*in1[p,x]; main out is a
            # throwaway broadcast AP.  (tensor_tensor_reduce would do the
            # same in one standard op but hangs TRN2 hardware.)
            dummy = dumps.tile([P, 1], f32, tag="dummy")
            return nc.vector.affine_mul_reduce(
                out=dummy.broadcast_to(in0.shape),
                accum_out=accum_col,
                in0=in0,
                in1=in1,
                scale=1.0,
                bias=0.0,
            )

        def rsqrt(invn_dst, ssq):
            # 1/max(sqrt(s), eps) == min(rsqrt(s), 1e12); Quake bit-trick
            # + 2 Newton iterations, all on DVE.
            n = ssq.shape[1]
            h = stats.tile([P, n], i32, tag="h")
            nc.vector.tensor_scalar(h[:], ssq.bitcast(i32), 1, None, op0=SHR)
            y = stats.tile([P, n], f32, tag="y")
            nc.vector.tensor_tensor(y[:].bitcast(i32), magic[:, :n], h[:], op=SUB)
            a = stats.tile([P, n], f32, tag="a")
            for _ in range(2):
                nc.vector.tensor_mul(a[:], y[:], y[:])
                nc.vector.tensor_mul(a[:], a[:], ssq)
                nc.vector.tensor_scalar(a[:], a[:], -0.5, 1.5, op0=MUL, op1=ADD)
                nc.vector.tensor_mul(y[:], y[:], a[:])
            nc.vector.tensor_scalar_min(invn_dst, y[:], 1.0e12)

        load_insts = []
        zbs = {}

        def load_group(g, chunks=2):
            # Pacing: one cast-DMA only sustains ~150 GB/s, so run two
            # chunks in parallel (~300 GB/s, near the 358 HBM cap) and
            # chain chunk n behind chunk n-2 to keep arrival order.
            # Group 0 uses four smaller chunks so its per-chunk prep
            # chain starts sooner.
            zb = zbpool.tile([P, GT, D], bf16, tag="zb")
            at = GT // chunks
            for s in range(chunks):
                ld = nc.gpsimd.dma_start(
                    out=zb[:, s * at : (s + 1) * at, :],
                    in_=fg[g][s * at * P : (s + 1) * at * P, :].rearrange(
                        "(a p) d -> p a d", p=P
                    ),
                )
                n = len(load_insts)
                if n >= 2:
                    add_dep_helper(
                        ld.ins, load_insts[n - 2].ins, reason="pace loads"
                    )
                load_insts.append(ld)
            zbs[g] = zb

        ft8s = {}

        ssqs = {}

        def prep_ssq(g, a, after=None):
            if g not in ssqs:
                ssqs[g] = stats.tile(
                    [P, GT], f32, tag=f"ssq{g % 2}", name=f"ssq_{g}"
                )
            ms = mulsum(zbs[g][:, a, :], zbs[g][:, a, :], ssqs[g][:, a : a + 1])
            if after is not None:
                # Hard ordering hint: without it the Tile scheduler's
                # cost model may slot this ssq ahead of the previous
                # group's scale/cast on the (FIFO, counter-semaphore) DVE
                # queue, inflating every downstream wait threshold.
                add_dep_helper(ms.ins, after.ins, reason="dve order")

        def prep_finish(g, two_queues=False):
            # rsqrt + row scale + transpose + fp8 cast for group g (the
            # 8 ssq mul-reduces were already emitted via prep_ssq).
            zb = zbs.pop(g)
            ssq = ssqs.pop(g)
            invn = stats.tile([P, GT], f32, tag="invn")
            rsqrt(invn[:], ssq[:])
            # Native 2-byte xbar transposes (one per row-tile a, keeping
            # both APs within the 2D-in/3D-out transpose constraint):
            # tb[dp, db, a*128+j] = zb[j, a, 128*db+dp], i.e. F^T with
            # d = 128*db + dp on the partition axis and columns ordered
            # like rows (a*128+j).  Each transpose is emitted right after
            # its row-tile's scale so it starts as early as possible.
            # During startup (ACT idle) they are split over both HWDGE
            # queues (SP + ACT) to halve the serial 8 x ~1.27us queue
            # time; in steady state ACT runs exps, so everything stays on
            # the SP queue.  The fp8 cast runs per column half so the
            # first matmuls only wait on 4 transposes.
            tb = tbpool.tile([P, DB, R], bf16, tag="tb")
            ft8 = f8pool.tile([P, DB, R], f8, tag=f"ft8_{g}", name=f"ft8_{g}")
            for half in range(2):
                for a in range(4 * half, 4 * half + 4):
                    nc.vector.tensor_scalar_mul(
                        zb[:, a, :], zb[:, a, :], invn[:, a : a + 1]
                    )
                    q = nc.scalar if (two_queues and a % 2 == 1) else nc.sync
                    q.dma_start(
                        out=tb[:, :, a * P : (a + 1) * P],
                        in_=zb[:, a, :],
                        transpose=True,
                    )
                sl = slice(half * 512, half * 512 + 512)
                nc.vector.tensor_copy(ft8[:, :, sl], tb[:, :, sl])
            ft8s[g] = ft8

        def prep_group0():
            # Startup-critical fast path for group 0: the load arrives in
            # four 2-row-tile chunks (2-wide ~300 GB/s), and each chunk
            # runs its ssq -> rsqrt-slice -> scale -> transpose chain the
            # moment it lands, with the fp8 cast per column half.  This
            # gets the first matmul going ~20us earlier than the serial
            # load-all-then-prep-all chain.
            zb = zbs.pop(0)
            ssq = stats.tile([P, GT], f32, tag="ssq0", name="ssq_0")
            invn = stats.tile([P, GT], f32, tag="invn")
            tb = tbpool.tile([P, DB, R], bf16, tag="tb")
            ft8 = f8pool.tile([P, DB, R], f8, tag="ft8_0", name="ft8_0")
            for c in range(4):
                for a in (2 * c, 2 * c + 1):
                    mulsum(zb[:, a, :], zb[:, a, :], ssq[:, a : a + 1])
                sl2 = slice(2 * c, 2 * c + 2)
                rsqrt(invn[:, sl2], ssq[:, sl2])
                for a in (2 * c, 2 * c + 1):
                    # Startup-only: the row scale runs on the otherwise
                    # idle ACT engine, keeping the DVE queue short so the
                    # transpose/cast chain is not pushed back by the
                    # scheduler's counter-semaphore thresholds.
                    nc.scalar.mul(zb[:, a, :], zb[:, a, :], invn[:, a : a + 1])
                    q = nc.scalar if a % 2 == 1 else nc.sync
                    q.dma_start(
                        out=tb[:, :, a * P : (a + 1) * P],
                        in_=zb[:, a, :],
                        transpose=True,
                    )
                if c % 2 == 1:
                    sl = slice((c // 2) * 512, (c // 2) * 512 + 512)
                    cast_i = nc.vector.tensor_copy(ft8[:, :, sl], tb[:, :, sl])
            ft8s[0] = ft8
            return cast_i

        def prep_group(g, two_queues=False, after=None):
            for a in range(GT):
                prep_ssq(g, a, after=after if a == 0 else None)
            prep_finish(g, two_queues)

        def sim_phase(g, prep_g=None):
            # prep_g: group prepped for use two phases later.  Its load
            # was issued three phases ahead, so the whole normalize/
            # transpose/cast chain is emitted up front here and runs
            # during this phase's PE/ACT work.
            if prep_g is not None:
                prep_group(prep_g)
            ft8g = ft8s[g]
            ft80 = ft8s[0]
            cs = None
            if g in (1, 2, 3):
                cs = cspool.tile([P, R], f32, tag="cs")
            e8 = None
            for r in range(GT):
                ps = psum.tile([P, R], f32, tag="ps")
                for h in range(H):
                    lhsT = ft80[:, 2 * h : 2 * h + 2, r * P : (r + 1) * P]
                    for ns in range(2):
                        nc.tensor.matmul(
                            ps[:, ns * 512 : (ns + 1) * 512],
                            lhsT,
                            ft8g[:, 2 * h : 2 * h + 2, ns * 512 : (ns + 1) * 512],
                            start=(h == 0),
                            stop=(h == H - 1),
                            perf_mode=DR,
                        )
                acc = sums_sb[:, g * GT + r : g * GT + r + 1]
                if g in (1, 2, 3):
                    if r % 2 == 0:
                        e8 = e8pool.tile([P, 2, R], f8, tag="e8")
                    nc.scalar.activation(
                        e8[:, r % 2, :], ps[:], EXPF, scale=1.0 / T, accum_out=acc
                    )
                    if r % 2 == 1:
                        pr = r // 2
                        for ns in range(2):
                            nc.tensor.matmul(
                                cs[0:1, ns * 512 : (ns + 1) * 512],
                                ones8[:, :, 0:1],
                                e8[:, :, ns * 512 : (ns + 1) * 512],
                                start=(pr == 0),
                                stop=(pr == GT // 2 - 1),
                                perf_mode=DR,
                            )
                else:
                    scr = scrpool.tile([P, R], bf16, tag="scr")
                    nc.scalar.activation(
                        scr[:], ps[:], EXPF, scale=1.0 / T, accum_out=acc
                    )
                    # Diag/pos extraction reads the bf16 exp SCRATCH (not
                    # PSUM): psum release then depends on the exp alone,
                    # and the extraction can lag freely (it only feeds the
                    # final host outputs).  bf16 rounding of the dominant
                    # e^{1/T} term leaves a +-0.4% residual on ~1% of the
                    # denominator - ~5e-4 relative on the loss after
                    # averaging, well inside the 2e-2 gate.  Host gets
                    # exp-domain values (see run()).
                    dst = diag_sb if g == 0 else pos_sb
                    mulsum(
                        scr[:, r * P : (r + 1) * P],
                        eye_sb[:],
                        dst[:, r : r + 1],
                    )
            if g in (1, 2, 3):
                nc.vector.tensor_copy(
                    csum_sb[0:1, (g - 1) * R : g * R], cs[0:1, :]
                )

        # Startup: groups 0 and 1 loaded+prepped before phase 0 (using
        # both HWDGE queues for the transposes while ACT is idle); later
        # groups' loads start immediately and their DVE prep is drip-fed
        # through the preceding phases.
        load_group(0, chunks=4)
        load_group(1)
        load_group(2)
        prep_group0()
        prep_group(1, two_queues=True)
        for g in range(NG):
            if g + 3 < NG:
                load_group(g + 3)
            sim_phase(g, prep_g=g + 2 if g + 2 < NG else None)

        nc.sync.dma_start(out=sums_out[:], in_=sums_sb[:])
        nc.sync.dma_start(out=diag_out[:], in_=diag_sb[:])
        nc.sync.dma_start(out=pos_out[:], in_=pos_sb[:])
        nc.sync.dma_start(out=csum_out[:, :], in_=csum_sb[0:1, :])

    nc.compile()
    return nc


def _get_nc():
    global _NC
    if _NC is None:
        _NC = _build()
    return _NC


def run(z1, z2, trace=False):
    """Run the SPMD kernel; returns (loss, BassKernelResults)."""
    from concourse.bass_utils import run_bass_kernel_spmd

    z1 = np.ascontiguousarray(z1, dtype=np.float32)
    z2 = np.ascontiguousarray(z2, dtype=np.float32)
    F = np.concatenate([z1, z2], axis=0)  # [8192, 512]
    eye_np = np.eye(P, dtype=np.float32)
    in_maps = []
    for c in range(NCORES):
        m = {"eye": eye_np}
        for k in range(NG):
            blk = (c + k) % G
            m[f"f{k}"] = F[blk * R : (blk + 1) * R]
        in_maps.append(m)
    res = run_bass_kernel_spmd(
        _get_nc(), in_maps, core_ids=list(range(NCORES)), trace=trace
    )
    e_diag_true = np.exp(1.0 / T)
    # Per-core row-major [1024] views; row i = rt*128 + p.
    RS, DG, PS, CSa = [], [], [], []
    for r in res.results:
        sums = r["sums"].astype(np.float64)  # [P, NG*GT]
        RS.append(sums.reshape(P, NG, GT).sum(axis=1).T.reshape(R))
        DG.append(r["diag"].astype(np.float64).T.reshape(R))
        PS.append(r["pos"].astype(np.float64).T.reshape(R))
        CSa.append(r["csum"].astype(np.float64).reshape(3, R))  # row g-1
    total = 0.0
    for b in range(G):
        # DG/PS hold exp-domain extractions: exp(diag/T), exp(pos/T).
        den = RS[b] - DG[b] + e_diag_true
        for g in (1, 2, 3):
            den = den + CSa[(b - g) % G][g - 1]
        total += (np.log(den) - np.log(PS[b])).sum()
    loss = total / (2.0 * B)
    return np.float32(loss), res


def kernel(z1, z2, labels=None, **_ignored):
    loss, _ = run(z1, z2, trace=False)
    return np.asarray(loss, dtype=np.float32)


if __name__ == "__main__":
    rng = np.random.default_rng(0)
    a = rng.standard_normal((B, D)).astype(np.float32)
    b = rng.standard_normal((B, D)).astype(np.float32)
    print(kernel(a, b, None))
